# revision 17
# baseline (speedup 1.0000x reference)
"""Trainium2 Bass kernel for a transformer decoder layer (self-attn +
cross-attn + FFN), data-parallel over batch across 8 NeuronCores.

kernel(**inputs) takes the FULL unsharded inputs (numpy, as in
reference.setup_inputs()) and returns the full (out, self_attn, enc_attn)
tuple. Inside: shard batch 256 -> 8 x 32, marshal layouts on host, run one
SPMD Bass kernel on cores 0-7, gather.

HW constraints baked in (found empirically):
- matmul operands must start at partition 0 (odd-head halves of qT/kT are
  partition-shifted down via SBUF->SBUF DMA);
- PSUM reads by ACT/DVE must start at partition 0;
- only one sync-wait per instruction (excess hoisted onto
  InstEventSemaphore);
- multiple matmuls into one PSUM bank: keep one row-band and uniform
  tile_position; transposes may share a bank as one accumulation group.
"""

import os

import numpy as np
import ml_dtypes

import concourse.bass as bass
import concourse.mybir as mybir
import concourse.tile as tile
from concourse.bass_utils import run_bass_kernel_spmd
from concourse.masks import make_identity

BF16 = ml_dtypes.bfloat16
F32 = mybir.dt.float32
BF = mybir.dt.bfloat16
AF = mybir.ActivationFunctionType
ALU = mybir.AluOpType

# ---- problem dims (per core) ----
NCORES = 8
B = 32            # local batches per core
LQ = 52
LK = 196
E = 512           # embed
D = 2048          # enc dim
H = 8
DK = 64
FF = 2048
T = B * LQ        # 1664 local dec tokens
S = B * LK        # 6272 local enc tokens
NEG = -1e9
EPS = 1e-5
SCALE = 0.125     # 1/sqrt(DK)

NT = T // 128     # 13 dec token tiles
ECH = E // 128    # 4 embed chunks
DCH = D // 128    # 16 enc-dim chunks
FCH = FF // 128   # 16 ff chunks
CB = 2            # batches per CA chunk
NCH = B // CB     # 16 CA chunks
S2 = CB * LK      # 392 enc rows per chunk

TCOLS = [(i * 512, min(512, T - i * 512)) for i in range((T + 511) // 512)]


def _split_waits(nc, max_waits=1):
    """This walrus accepts only one sync-wait per instruction; hoist excess
    waits onto standalone event-semaphore instructions just before."""
    n_new = 0
    for bb in nc.main_func.blocks:
        new_list = []
        for ins in bb.instructions:
            w = ins.sync_info.on_wait if ins.sync_info else None
            if w and len(w) > max_waits:
                waits = list(w)
                while len(waits) > max_waits:
                    chunk, waits = waits[:max_waits], waits[max_waits:]
                    nd = mybir.InstEventSemaphore(
                        name=f"I-ws-{n_new}", ins=[], outs=[])
                    nd.engine = ins.engine
                    nd.sync_info = mybir.SyncInfo(on_wait=chunk, on_update=[])
                    nc.register_instruction(nd)
                    new_list.append(nd)
                    n_new += 1
                ins.sync_info = mybir.SyncInfo(
                    on_wait=waits, on_update=list(ins.sync_info.on_update or [])
                )
            new_list.append(ins)
        bb.instructions[:] = new_list
    return n_new


def build_nc():
    stages = os.environ.get("K_STAGES", "123")
    nc = bass.Bass()

    # ---------------- DRAM I/O ----------------
    d_xT = nc.dram_tensor("xT", [ECH, 128, T], BF, kind="ExternalInput")
    d_xnat = nc.dram_tensor("xnat", [NT, 128, E], F32, kind="ExternalInput")
    d_enc = nc.dram_tensor("encT", [NCH, DCH, 128, S2], BF, kind="ExternalInput")
    d_bsa = nc.dram_tensor("bias_sa", [LQ, B * LQ], BF, kind="ExternalInput")
    d_bca = nc.dram_tensor("bias_ca", [LQ, B * LK], BF, kind="ExternalInput")

    d_sa_wq = nc.dram_tensor("sa_wq", [ECH, 128, E], BF, kind="ExternalInput")
    d_sa_wk = nc.dram_tensor("sa_wk", [ECH, 128, E], BF, kind="ExternalInput")
    d_sa_wv = nc.dram_tensor("sa_wv", [ECH, 128, E], BF, kind="ExternalInput")
    d_sa_wo = nc.dram_tensor("sa_wo", [ECH, 128, E], BF, kind="ExternalInput")
    d_ca_wq = nc.dram_tensor("ca_wq", [ECH, 128, E], BF, kind="ExternalInput")
    d_ca_wk = nc.dram_tensor("ca_wk", [DCH, 128, E], BF, kind="ExternalInput")
    d_ca_wv = nc.dram_tensor("ca_wv", [DCH, 128, E], BF, kind="ExternalInput")
    d_ca_wo = nc.dram_tensor("ca_wo", [ECH, 128, E], BF, kind="ExternalInput")
    d_ff_w1 = nc.dram_tensor("ff_w1", [ECH, 128, FF], BF, kind="ExternalInput")
    d_ff_w2 = nc.dram_tensor("ff_w2", [FCH, 128, E], BF, kind="ExternalInput")

    # packed per-partition biases [128, ntiles] f32
    d_sa_bq = nc.dram_tensor("sa_bq_p", [128, ECH], F32, kind="ExternalInput")
    d_sa_bk = nc.dram_tensor("sa_bk_p", [128, ECH], F32, kind="ExternalInput")
    d_ca_bq = nc.dram_tensor("ca_bq_p", [128, ECH], F32, kind="ExternalInput")
    d_ca_bk = nc.dram_tensor("ca_bk_p", [128, ECH], F32, kind="ExternalInput")
    d_ff_b1 = nc.dram_tensor("ff_b1_p", [128, FCH], F32, kind="ExternalInput")
    # row biases (bf16, applied via K=1 matmul); *_bo already fold bv@Wo
    d_sa_bo = nc.dram_tensor("sa_bo_r", [1, E], BF, kind="ExternalInput")
    d_ca_bo = nc.dram_tensor("ca_bo_r", [1, E], BF, kind="ExternalInput")
    d_ff_b2 = nc.dram_tensor("ff_b2_r", [1, E], BF, kind="ExternalInput")

    d_out = nc.dram_tensor("out_t", [NT, 128, E], F32, kind="ExternalOutput")
    d_asa = nc.dram_tensor("attn_sa", [B, LQ, H * LQ], F32, kind="ExternalOutput")
    d_aca = nc.dram_tensor("attn_ca", [B, LQ, H * LK], F32, kind="ExternalOutput")
    d_x2 = nc.dram_tensor("x2_spill", [NT, 128, E], F32)
    d_x3 = nc.dram_tensor("x3_spill", [NT, 128, E], F32)

    with tile.TileContext(nc) as tc:
        with (
            tc.tile_pool(name="const", bufs=1) as cpool,
            tc.tile_pool(name="glob", bufs=1) as gpool,
        ):
            # ---- constants ----
            id_f = cpool.tile([128, 128], F32, name="id_f")
            make_identity(nc, id_f[:])
            id_b = cpool.tile([128, 128], BF, name="id_b")
            make_identity(nc, id_b[:])
            ones_r = cpool.tile([1, 128], BF, name="ones_r")
            nc.gpsimd.memset(ones_r[:], 1.0)
            eps_c = cpool.tile([128, 1], F32, name="eps_c")
            nc.gpsimd.memset(eps_c[:], EPS)

            def load_const(name, dram, shape, dt):
                t = cpool.tile(shape, dt, name=name)
                nc.sync.dma_start(out=t[:], in_=dram[:, :])
                return t

            sa_bq = load_const("sa_bq", d_sa_bq, [128, ECH], F32)
            sa_bk = load_const("sa_bk", d_sa_bk, [128, ECH], F32)
            ca_bq = load_const("ca_bq", d_ca_bq, [128, ECH], F32)
            ca_bk = load_const("ca_bk", d_ca_bk, [128, ECH], F32)
            ff_b1 = load_const("ff_b1", d_ff_b1, [128, FCH], F32)
            sa_bo = load_const("sa_bo", d_sa_bo, [1, E], BF)
            ca_bo = load_const("ca_bo", d_ca_bo, [1, E], BF)
            ff_b2 = load_const("ff_b2", d_ff_b2, [1, E], BF)
            bsa = load_const("bsa", d_bsa, [LQ, B * LQ], BF)
            bca = load_const("bca", d_bca, [LQ, B * LK], BF)

            x2T = [gpool.tile([128, T], BF, name=f"x2T_{j}") for j in range(ECH)]

            def qkT_gemm(psum_pool, w_sb, src_T, dst, dst_odd, bias, ncols):
                """dst[dt][128, ncols] = (W.T @ srcT) + bias (bf16), then
                partition-shift rows 64:128 into dst_odd[dt][0:64]."""
                nch = len(w_sb)
                cols = [(i * 512, min(512, ncols - i * 512))
                        for i in range((ncols + 511) // 512)]
                for dt in range(ECH):
                    for (c0, cw) in cols:
                        ps = psum_pool.tile([128, 512], F32, tag="mm", bufs=2,
                                            name="ps_qk")
                        for j in range(nch):
                            nc.tensor.matmul(
                                ps[:, 0:cw],
                                lhsT=w_sb[j][:, dt * 128:(dt + 1) * 128],
                                rhs=src_T[j][:, c0:c0 + cw],
                                start=(j == 0), stop=(j == nch - 1),
                            )
                        nc.scalar.activation(
                            dst[dt][:, c0:c0 + cw], ps[:, 0:cw], AF.Identity,
                            bias=bias[:, dt:dt + 1],
                        )
                    nc.sync.dma_start(out=dst_odd[dt][0:64, :],
                                      in_=dst[dt][64:128, :])

            def ln_tile(psum_ap, res_ap, dst_ap, pool, pfx):
                """dst = LN(psum + res) for one [128, E] tile."""
                y = pool.tile([128, E], F32, tag=f"{pfx}y", bufs=3, name="ln_y")
                nc.vector.tensor_add(y[:], psum_ap, res_ap)
                st = pool.tile([128, 6], F32, tag=f"{pfx}st", bufs=3, name="ln_st")
                nc.vector.bn_stats(st[:], y[:])
                mv = pool.tile([128, 2], F32, tag=f"{pfx}mv", bufs=3, name="ln_mv")
                nc.vector.bn_aggr(mv[:], st[:])
                sd = pool.tile([128, 1], F32, tag=f"{pfx}sd", bufs=3, name="ln_sd")
                nc.scalar.activation(sd[:], mv[:, 1:2], AF.Sqrt, bias=eps_c[:])
                rs = pool.tile([128, 1], F32, tag=f"{pfx}rs", bufs=3, name="ln_rs")
                nc.vector.reciprocal(rs[:], sd[:])
                nc.gpsimd.tensor_scalar(
                    dst_ap, y[:], mv[:, 0:1], rs[:],
                    op0=ALU.subtract, op1=ALU.mult,
                )

            def oproj_ln(psum_pool, ctxT_all, w_sb, bo_row, d_res, d_dst,
                         work_pool, pfx, dstT=None):
                """LN(ctx@Wo + bo + res[tt]) streamed dram->dram; ctxT_all is
                [64, H*T] (contraction in 8 chunks of 64); w_sb is 8 [64, E]
                head-chunk weight tiles. Optionally write transposed bf16
                copy into dstT."""
                cv = ctxT_all[:].rearrange("p (h t) -> p h t", h=H)
                for tt in range(NT):
                    ps = psum_pool.tile([128, 512], F32, tag="mm", bufs=2,
                                        name="ps_o")
                    for h in range(H):
                        nc.tensor.matmul(
                            ps[:], lhsT=cv[0:64, h, tt * 128:(tt + 1) * 128],
                            rhs=w_sb[h][0:64, :], start=(h == 0), stop=False,
                        )
                    nc.tensor.matmul(ps[:], lhsT=ones_r[0:1, :],
                                     rhs=bo_row[0:1, :], start=False, stop=True)
                    res = work_pool.tile([128, E], F32, tag=f"{pfx}res", bufs=3,
                                         name="res_in")
                    nc.sync.dma_start(out=res[:], in_=d_res[tt])
                    dst = work_pool.tile([128, E], F32, tag=f"{pfx}dst", bufs=3,
                                         name="dst_t")
                    ln_tile(ps[:], res[:], dst[:], work_pool, pfx)
                    nc.sync.dma_start(out=d_dst[tt], in_=dst[:])
                    if dstT is not None:
                        px = psum_pool.tile([128, 512], F32, tag="px", bufs=1,
                                            name="ps_px")
                        for j in range(ECH):
                            nc.tensor.matmul(
                                px[:, j * 128:(j + 1) * 128],
                                lhsT=dst[:, j * 128:(j + 1) * 128],
                                rhs=id_f[:, :], is_transpose=True,
                                start=(j == 0), stop=(j == ECH - 1),
                            )
                        for j in range(ECH):
                            nc.vector.tensor_copy(
                                dstT[j][:, tt * 128:(tt + 1) * 128],
                                px[:, j * 128:(j + 1) * 128],
                            )

            # ================= Stage 1: self-attention =================
            if "1" in stages:
             with (
                tc.tile_pool(name="s1", bufs=1) as p1,
                tc.tile_pool(name="ps1", bufs=1, space="PSUM") as pp1,
             ):
                ctxT = p1.tile([64, H * T], BF, name="ctxT")
                ctxv = ctxT[:].rearrange("p (h t) -> p h t", h=H)

                with tc.tile_pool(name="s1a", bufs=1) as p1a:
                    qT = [p1a.tile([128, T], BF, name=f"qT_{j}")
                          for j in range(ECH)]
                    kT = [p1a.tile([128, T], BF, name=f"kT_{j}")
                          for j in range(ECH)]
                    qTo = [p1a.tile([64, T], BF, name=f"qTo_{j}")
                           for j in range(ECH)]
                    kTo = [p1a.tile([64, T], BF, name=f"kTo_{j}")
                           for j in range(ECH)]
                    vsb = [p1a.tile([LQ, E], BF, name=f"v_{t}")
                           for t in range(B)]

                    with tc.tile_pool(name="s1x", bufs=1) as p1x:
                        wq = [p1x.tile([128, E], BF, name=f"sa_wq_{j}")
                              for j in range(ECH)]
                        wk = [p1x.tile([128, E], BF, name=f"sa_wk_{j}")
                              for j in range(ECH)]
                        wv = [p1x.tile([128, E], BF, name=f"sa_wv_{j}")
                              for j in range(ECH)]
                        xT = [p1x.tile([128, T], BF, name=f"xT_{j}")
                              for j in range(ECH)]
                        for j in range(ECH):
                            nc.sync.dma_start(out=wq[j][:], in_=d_sa_wq[j])
                            nc.sync.dma_start(out=wk[j][:], in_=d_sa_wk[j])
                            nc.sync.dma_start(out=wv[j][:], in_=d_sa_wv[j])
                            nc.sync.dma_start(out=xT[j][:], in_=d_xT[j])
                        qkT_gemm(pp1, wq, xT, qT, qTo, sa_bq, T)
                        qkT_gemm(pp1, wk, xT, kT, kTo, sa_bk, T)
                        # v: natural layout; one [52, E] tile per batch
                        for b_ in range(B):
                            ps = pp1.tile([LQ, 512], F32, tag="mm", bufs=2,
                                          name="ps_v")
                            for j in range(ECH):
                                nc.tensor.matmul(
                                    ps[:],
                                    lhsT=xT[j][:, b_ * LQ:(b_ + 1) * LQ],
                                    rhs=wv[j][:], start=(j == 0),
                                    stop=(j == ECH - 1),
                                )
                            nc.scalar.copy(vsb[b_][:], ps[:])

                    # ---- per-batch attention ----
                    nb_attn = int(os.environ.get("K_S1NB", str(B)))
                    for b in range(nb_attn):
                        ps_s = pp1.tile([LQ, 512], F32, tag="ssa", bufs=2,
                                        name="ps_s")
                        nc.tensor.matmul(
                            ps_s[:, 0:H * LQ],
                            lhsT=id_b[0:LQ, 0:LQ],
                            rhs=bsa[:, b * LQ:(b + 1) * LQ].unsqueeze(1)
                                .broadcast_to((LQ, H, LQ)),
                            start=True, stop=False,
                        )
                        for h in range(H):
                            qsrc = (qT if h % 2 == 0 else qTo)[h // 2]
                            ksrc = (kT if h % 2 == 0 else kTo)[h // 2]
                            sl = slice(b * LQ, (b + 1) * LQ)
                            nc.tensor.matmul(
                                ps_s[:, LQ * h:LQ * (h + 1)],
                                lhsT=qsrc[0:DK, sl], rhs=ksrc[0:DK, sl],
                                start=False, stop=(h == H - 1),
                            )
                        aexp = p1a.tile([LQ, H * LQ], F32, tag="aexp", bufs=2,
                                        name="aexp")
                        nc.scalar.activation(aexp[:], ps_s[:, 0:H * LQ],
                                             AF.Exp, scale=SCALE)
                        sums = p1a.tile([LQ, H], F32, tag="sums", bufs=2,
                                        name="sums")
                        nc.vector.reduce_sum(
                            sums[:], aexp[:].rearrange("p (h n) -> p h n", h=H),
                            axis=mybir.AxisListType.X,
                        )
                        rcp = p1a.tile([LQ, H], F32, tag="rcp", bufs=2,
                                       name="rcp")
                        nc.vector.reciprocal(rcp[:], sums[:])
                        attn = p1a.tile([LQ, H * LQ], F32, tag="attn", bufs=2,
                                        name="attn")
                        nc.gpsimd.tensor_tensor(
                            attn[:].rearrange("p (h n) -> p h n", h=H),
                            aexp[:].rearrange("p (h n) -> p h n", h=H),
                            rcp[:].unsqueeze(2).broadcast_to((LQ, H, LQ)),
                            op=ALU.mult,
                        )
                        nc.sync.dma_start(out=d_asa[b], in_=attn[:])
                        # transpose attn -> [n, m] per head (one group/bank)
                        ps_t = pp1.tile([LQ, 512], F32, tag="tsa", bufs=1,
                                        name="ps_t")
                        for h in range(H):
                            nc.tensor.matmul(
                                ps_t[:, h * LQ:(h + 1) * LQ],
                                lhsT=attn[:, h * LQ:(h + 1) * LQ],
                                rhs=id_f[0:LQ, 0:LQ], is_transpose=True,
                                start=(h == 0), stop=(h == H - 1),
                            )
                        attnT = p1a.tile([LQ, H * LQ], BF, tag="attnT", bufs=2,
                                         name="attnT")
                        nc.vector.tensor_copy(attnT[:], ps_t[:, 0:H * LQ])
                        # ctx: single row band [64, 512], head h at col 52h,
                        # separate single-MM groups
                        ps_c = pp1.tile([64, 512], F32, tag="csa", bufs=2,
                                        name="ps_c")
                        for h in range(H):
                            nc.tensor.matmul(
                                ps_c[0:64, LQ * h:LQ * (h + 1)],
                                lhsT=vsb[b][:, DK * h:DK * (h + 1)],
                                rhs=attnT[:, h * LQ:(h + 1) * LQ],
                                start=True, stop=True,
                            )
                        nc.vector.tensor_copy(
                            ctxv[0:64, :, b * LQ:(b + 1) * LQ],
                            ps_c[0:64, 0:H * LQ]
                            .rearrange("p (h n) -> p h n", h=H),
                        )

                if os.environ.get("K_S1OPROJ", "1") == "1":
                    with tc.tile_pool(name="s1c", bufs=1) as p1c:
                        wo = [p1c.tile([64, E], BF, name=f"sa_wo_{h}")
                              for h in range(H)]
                        for h in range(H):
                            nc.sync.dma_start(
                                out=wo[h][0:64, :],
                                in_=d_sa_wo[h // 2, 64 * (h % 2):
                                            64 * (h % 2) + 64])
                        oproj_ln(pp1, ctxT, wo, sa_bo, d_xnat, d_x2, p1c,
                                 "ln1", dstT=x2T)

            # ================= Stage 2: cross-attention =================
            if "2" in stages:
             with (
                tc.tile_pool(name="s2", bufs=1) as p2,
                tc.tile_pool(name="ps2", bufs=1, space="PSUM") as pp2,
             ):
                qTc = [p2.tile([128, T], BF, name=f"qTc_{j}") for j in range(ECH)]
                qTco = [p2.tile([64, T], BF, name=f"qTco_{j}")
                        for j in range(ECH)]
                ctxTc = p2.tile([64, H * T], BF, name="ctxTc")
                ctxcv = ctxTc[:].rearrange("p (h t) -> p h t", h=H)

                with tc.tile_pool(name="s2a", bufs=1) as p2a:
                    cwq = [p2a.tile([128, E], BF, name=f"ca_wq_{j}")
                           for j in range(ECH)]
                    for j in range(ECH):
                        nc.sync.dma_start(out=cwq[j][:], in_=d_ca_wq[j])
                    qkT_gemm(pp2, cwq, x2T, qTc, qTco, ca_bq, T)

                with tc.tile_pool(name="s2b", bufs=1) as p2b:
                    cwk = [p2b.tile([128, E], BF, name=f"ca_wk_{j}")
                           for j in range(DCH)]
                    cwv = [p2b.tile([128, E], BF, name=f"ca_wv_{j}")
                           for j in range(DCH)]
                    for j in range(DCH):
                        nc.sync.dma_start(out=cwk[j][:], in_=d_ca_wk[j])
                        nc.sync.dma_start(out=cwv[j][:], in_=d_ca_wv[j])

                    for c in range(NCH):
                        encs = [p2b.tile([128, S2], BF, tag=f"enc{j}", bufs=2,
                                         name=f"enc_{j}") for j in range(DCH)]
                        for j in range(DCH):
                            nc.sync.dma_start(out=encs[j][:], in_=d_enc[c, j])
                        kTc = [p2b.tile([128, S2], BF, tag=f"kTc{dt}", bufs=2,
                                        name=f"kTc_{dt}") for dt in range(ECH)]
                        kTco = [p2b.tile([64, S2], BF, tag=f"kTco{dt}", bufs=2,
                                         name=f"kTco_{dt}")
                                for dt in range(ECH)]
                        for dt in range(ECH):
                            ps = pp2.tile([128, 512], F32, tag="mm", bufs=2,
                                          name="ps_kc")
                            for j in range(DCH):
                                nc.tensor.matmul(
                                    ps[:, 0:S2],
                                    lhsT=cwk[j][:, dt * 128:(dt + 1) * 128],
                                    rhs=encs[j][:], start=(j == 0),
                                    stop=(j == DCH - 1),
                                )
                            nc.scalar.activation(kTc[dt][:], ps[:, 0:S2],
                                                 AF.Identity,
                                                 bias=ca_bk[:, dt:dt + 1])
                            nc.sync.dma_start(out=kTco[dt][0:64, :],
                                              in_=kTc[dt][64:128, :])
                        # v for this chunk: per batch [128,512]+[68,512]
                        vc = []
                        for bi in range(CB):
                            for (r0, rw) in ((0, 128), (128, LK - 128)):
                                ps = pp2.tile([128, 512], F32, tag="mm",
                                              bufs=2, name="ps_vc")
                                for j in range(DCH):
                                    nc.tensor.matmul(
                                        ps[0:rw, :],
                                        lhsT=encs[j][:, bi * LK + r0:
                                                     bi * LK + r0 + rw],
                                        rhs=cwv[j][:], start=(j == 0),
                                        stop=(j == DCH - 1),
                                    )
                                vt_ = p2b.tile([128, E], BF,
                                               tag=f"vc{bi}_{r0}", bufs=2,
                                               name=f"vc_{bi}_{r0}")
                                nc.vector.tensor_copy(vt_[0:rw, :],
                                                      ps[0:rw, :])
                                vc.append(vt_)

                        for bi in range(CB):
                            b = c * CB + bi
                            # scores: 4 psum tiles, 2 heads each, dense 392
                            pss = [pp2.tile([LQ, 512], F32, tag="sca", bufs=3,
                                            name="ps_sc") for _ in range(4)]
                            for t_ in range(4):
                                nc.tensor.matmul(
                                    pss[t_][:, 0:2 * LK],
                                    lhsT=id_b[0:LQ, 0:LQ],
                                    rhs=bca[:, b * LK:(b + 1) * LK]
                                        .unsqueeze(1)
                                        .broadcast_to((LQ, 2, LK)),
                                    start=True, stop=False,
                                )
                                for hh in range(2):
                                    h = 2 * t_ + hh
                                    qs = (qTc if h % 2 == 0 else qTco)[h // 2]
                                    ks = (kTc if h % 2 == 0 else kTco)[h // 2]
                                    nc.tensor.matmul(
                                        pss[t_][:, LK * hh:LK * (hh + 1)],
                                        lhsT=qs[0:DK, b * LQ:(b + 1) * LQ],
                                        rhs=ks[0:DK, bi * LK:(bi + 1) * LK],
                                        start=False, stop=(hh == 1),
                                    )
                            aexp = p2b.tile([LQ, H * LK], F32, tag="aexpc",
                                            bufs=2, name="aexpc")
                            for t_ in range(4):
                                nc.scalar.activation(
                                    aexp[:, t_ * 2 * LK:(t_ + 1) * 2 * LK],
                                    pss[t_][:, 0:2 * LK], AF.Exp, scale=SCALE,
                                )
                            sums = p2b.tile([LQ, H], F32, tag="sumsc", bufs=2,
                                            name="sumsc")
                            nc.vector.reduce_sum(
                                sums[:],
                                aexp[:].rearrange("p (h n) -> p h n", h=H),
                                axis=mybir.AxisListType.X,
                            )
                            rcp = p2b.tile([LQ, H], F32, tag="rcpc", bufs=2,
                                           name="rcpc")
                            nc.vector.reciprocal(rcp[:], sums[:])
                            attn = p2b.tile([LQ, H * LK], F32, tag="attnc",
                                            bufs=2, name="attnc")
                            nc.gpsimd.tensor_tensor(
                                attn[:].rearrange("p (h n) -> p h n", h=H),
                                aexp[:].rearrange("p (h n) -> p h n", h=H),
                                rcp[:].unsqueeze(2).broadcast_to((LQ, H, LK)),
                                op=ALU.mult,
                            )
                            nc.sync.dma_start(out=d_aca[b], in_=attn[:])
                            # transposes: per head 2 chunks (128 + 68 rows);
                            # one bank per 4 heads: top cols 0:208, bottom
                            # rows 0:68 cols 208:416 — one group per bank
                            atop = p2b.tile([128, H * LQ], BF, tag="atop",
                                            bufs=2, name="atop")
                            abot = p2b.tile([128, H * LQ], BF, tag="abot",
                                            bufs=2, name="abot")
                            for g in range(2):
                                pt = pp2.tile([128, 512], F32, tag="tca",
                                              bufs=2, name="pt")
                                # group must open and close with full-height
                                # (128-row) writes so every row range closes
                                nc.tensor.matmul(
                                    pt[:, 0:LQ],
                                    lhsT=attn[:, (4 * g) * LK:
                                              (4 * g) * LK + 128],
                                    rhs=id_f[0:LQ, 0:LQ], is_transpose=True,
                                    start=True, stop=False,
                                )
                                for hh in range(4):
                                    h = 4 * g + hh
                                    nc.tensor.matmul(
                                        pt[0:LK - 128,
                                           4 * LQ + hh * LQ:
                                           4 * LQ + (hh + 1) * LQ],
                                        lhsT=attn[:, h * LK + 128:
                                                  (h + 1) * LK],
                                        rhs=id_f[0:LQ, 0:LQ],
                                        is_transpose=True,
                                        start=False, stop=False,
                                    )
                                for hh in range(1, 4):
                                    h = 4 * g + hh
                                    nc.tensor.matmul(
                                        pt[:, hh * LQ:(hh + 1) * LQ],
                                        lhsT=attn[:, h * LK:h * LK + 128],
                                        rhs=id_f[0:LQ, 0:LQ],
                                        is_transpose=True,
                                        start=False, stop=(hh == 3),
                                    )
                                nc.vector.tensor_copy(
                                    atop[:, g * 4 * LQ:(g + 1) * 4 * LQ],
                                    pt[:, 0:4 * LQ])
                                nc.vector.tensor_copy(
                                    abot[0:LK - 128,
                                         g * 4 * LQ:(g + 1) * 4 * LQ],
                                    pt[0:LK - 128, 4 * LQ:8 * LQ])
                            # ctx: single row band [64, 512], head h at col
                            # 52h, 2-MM accumulation group per head
                            ps_cc = pp2.tile([64, 512], F32, tag="cca",
                                             bufs=1, name="ps_cc")
                            for h in range(H):
                                nc.tensor.matmul(
                                    ps_cc[0:64, LQ * h:LQ * (h + 1)],
                                    lhsT=vc[2 * bi][0:128,
                                                    DK * h:DK * (h + 1)],
                                    rhs=atop[:, h * LQ:(h + 1) * LQ],
                                    start=True, stop=False,
                                )
                                nc.tensor.matmul(
                                    ps_cc[0:64, LQ * h:LQ * (h + 1)],
                                    lhsT=vc[2 * bi + 1][0:LK - 128,
                                                        DK * h:DK * (h + 1)],
                                    rhs=abot[0:LK - 128,
                                             h * LQ:(h + 1) * LQ],
                                    start=False, stop=True,
                                )
                            nc.vector.tensor_copy(
                                ctxcv[0:64, :, b * LQ:(b + 1) * LQ],
                                ps_cc[0:64, 0:H * LQ]
                                .rearrange("p (h n) -> p h n", h=H),
                            )

                with tc.tile_pool(name="s2c", bufs=1) as p2c:
                    cwo = [p2c.tile([64, E], BF, name=f"ca_wo_{h}")
                           for h in range(H)]
                    for h in range(H):
                        nc.sync.dma_start(
                            out=cwo[h][0:64, :],
                            in_=d_ca_wo[h // 2, 64 * (h % 2):
                                        64 * (h % 2) + 64])
                    oproj_ln(pp2, ctxTc, cwo, ca_bo, d_x2, d_x3, p2c, "ln2")

            # ================= Stage 3: FFN =================
            if "3" in stages:
             with (
                tc.tile_pool(name="s3", bufs=1) as p3,
                tc.tile_pool(name="ps3", bufs=1, space="PSUM") as pp3,
             ):
                x3 = [p3.tile([128, E], F32, name=f"x3_{i}") for i in range(NT)]
                x3T = [p3.tile([128, T], BF, name=f"x3T_{j}")
                       for j in range(ECH)]
                for tt in range(NT):
                    nc.sync.dma_start(out=x3[tt][:], in_=d_x3[tt])
                    px = pp3.tile([128, 512], F32, tag="px", bufs=2, name="px3")
                    for j in range(ECH):
                        nc.tensor.matmul(
                            px[:, j * 128:(j + 1) * 128],
                            lhsT=x3[tt][:, j * 128:(j + 1) * 128],
                            rhs=id_f[:, :], is_transpose=True,
                            start=(j == 0), stop=(j == ECH - 1),
                        )
                    for j in range(ECH):
                        nc.vector.tensor_copy(
                            x3T[j][:, tt * 128:(tt + 1) * 128],
                            px[:, j * 128:(j + 1) * 128])

                w1 = [p3.tile([128, FF], BF, name=f"ff_w1_{j}") for j in range(ECH)]
                w2 = [p3.tile([128, E], BF, name=f"ff_w2_{j}") for j in range(FCH)]
                for j in range(ECH):
                    nc.sync.dma_start(out=w1[j][:], in_=d_ff_w1[j])
                for j in range(FCH):
                    nc.sync.dma_start(out=w2[j][:], in_=d_ff_w2[j])

                hT = [p3.tile([128, T], BF, name=f"hT_{j}") for j in range(FCH)]
                for ft in range(FCH):
                    for (c0, cw) in TCOLS:
                        ps = pp3.tile([128, 512], F32, tag="mm", bufs=3,
                                      name="ps_h")
                        for j in range(ECH):
                            nc.tensor.matmul(
                                ps[:, 0:cw],
                                lhsT=w1[j][:, ft * 128:(ft + 1) * 128],
                                rhs=x3T[j][:, c0:c0 + cw],
                                start=(j == 0), stop=(j == ECH - 1),
                            )
                        nc.scalar.activation(
                            hT[ft][:, c0:c0 + cw], ps[:, 0:cw], AF.Relu,
                            bias=ff_b1[:, ft:ft + 1],
                        )

                for tt in range(NT):
                    ps = pp3.tile([128, 512], F32, tag="mm", bufs=3, name="ps_f")
                    for j in range(FCH):
                        nc.tensor.matmul(
                            ps[:], lhsT=hT[j][:, tt * 128:(tt + 1) * 128],
                            rhs=w2[j][:], start=(j == 0), stop=False,
                        )
                    nc.tensor.matmul(ps[:], lhsT=ones_r[0:1, :],
                                     rhs=ff_b2[0:1, :], start=False, stop=True)
                    of = p3.tile([128, E], F32, tag="fout", bufs=3, name="fout")
                    ln_tile(ps[:], x3[tt][:], of[:], p3, "ln3")
                    nc.sync.dma_start(out=d_out[tt], in_=of[:])

    _split_waits(nc)
    return nc


_NC_CACHE = None


def _get_nc():
    global _NC_CACHE
    if _NC_CACHE is None:
        _NC_CACHE = build_nc()
    return _NC_CACHE


def _marshal_core(dec, enc, mask_sa, mask_ca, w):
    """Per-core input map. dec [32,52,512] f32, enc [32,196,2048] f32."""
    X = np.ascontiguousarray(dec, np.float32).reshape(T, E)
    m = {}
    m["xnat"] = np.ascontiguousarray(X.reshape(NT, 128, E))
    m["xT"] = np.ascontiguousarray(X.T).astype(BF16).reshape(ECH, 128, T)
    encS = np.ascontiguousarray(enc, np.float32).reshape(S, D)
    encT = np.ascontiguousarray(encS.T).astype(BF16)   # [2048, 6272]
    m["encT"] = np.ascontiguousarray(
        encT.reshape(DCH, 128, NCH, S2).transpose(2, 0, 1, 3))
    m["bias_sa"] = np.ascontiguousarray(
        np.where(mask_sa, np.float32(NEG), np.float32(0.0))
        .transpose(1, 0, 2).reshape(LQ, B * LQ)).astype(BF16)
    m["bias_ca"] = np.ascontiguousarray(
        np.where(mask_ca, np.float32(NEG), np.float32(0.0))
        .transpose(1, 0, 2).reshape(LQ, B * LK)).astype(BF16)
    m.update(w)
    return m


def _marshal_weights(i):
    """Shared (replicated) weight marshalling; i = full inputs dict."""
    w = {}

    def f32(x):
        return np.asarray(x, np.float32)

    def wt(name, arr, nch):
        w[name] = np.ascontiguousarray(
            f32(arr).astype(BF16).reshape(nch, 128, -1))

    wt("sa_wq", i["sa_Wq"], ECH)
    wt("sa_wk", i["sa_Wk"], ECH)
    wt("sa_wv", i["sa_Wv"], ECH)
    wt("sa_wo", i["sa_Wo"], ECH)
    wt("ca_wq", i["ca_Wq"], ECH)
    wt("ca_wk", i["ca_Wk"], DCH)
    wt("ca_wv", i["ca_Wv"], DCH)
    wt("ca_wo", i["ca_Wo"], ECH)
    wt("ff_w1", i["ff_W1"], ECH)
    wt("ff_w2", i["ff_W2"], FCH)

    def bp(name, arr, nch):
        w[name] = np.ascontiguousarray(f32(arr).reshape(nch, 128).T)

    bp("sa_bq_p", i["sa_bq"], ECH)
    bp("sa_bk_p", i["sa_bk"], ECH)
    bp("ca_bq_p", i["ca_bq"], ECH)
    bp("ca_bk_p", i["ca_bk"], ECH)
    bp("ff_b1_p", i["ff_b1"], FCH)
    # fold v-bias through the output projection: attn rows sum to 1, so
    # ctx@Wo with v = XWv + bv equals (attn@XWv)@Wo + bv@Wo
    sa_bo_eff = f32(i["sa_bo"]) + f32(i["sa_bv"]) @ f32(i["sa_Wo"])
    ca_bo_eff = f32(i["ca_bo"]) + f32(i["ca_bv"]) @ f32(i["ca_Wo"])
    w["sa_bo_r"] = sa_bo_eff.astype(BF16).reshape(1, E)
    w["ca_bo_r"] = ca_bo_eff.astype(BF16).reshape(1, E)
    w["ff_b2_r"] = f32(i["ff_b2"]).astype(BF16).reshape(1, E)
    return w


def kernel(**inputs):
    inputs = {k: np.asarray(v) for k, v in inputs.items()}
    w = _marshal_weights(inputs)
    dec = np.asarray(inputs["dec_inputs"], np.float32)
    enc = np.asarray(inputs["enc_outputs"], np.float32)
    msa = np.asarray(inputs["dec_self_attn_mask"], bool)
    mca = np.asarray(inputs["dec_enc_attn_mask"], bool)

    in_maps = []
    for c in range(NCORES):
        s = slice(c * B, (c + 1) * B)
        in_maps.append(_marshal_core(dec[s], enc[s], msa[s], mca[s], w))

    nc = _get_nc()
    res = run_bass_kernel_spmd(nc, in_maps, core_ids=list(range(NCORES)))

    outs, asas, acas = [], [], []
    for c in range(NCORES):
        r = res.results[c]
        outs.append(r["out_t"].reshape(B, LQ, E))
        asas.append(r["attn_sa"].reshape(B, LQ, H, LQ).transpose(0, 2, 1, 3))
        acas.append(r["attn_ca"].reshape(B, LQ, H, LK).transpose(0, 2, 1, 3))
    out = np.concatenate(outs, 0)
    self_attn = np.ascontiguousarray(np.concatenate(asas, 0))
    enc_attn = np.ascontiguousarray(np.concatenate(acas, 0))
    return out, self_attn, enc_attn
